# revision 2
# baseline (speedup 1.0000x reference)
"""Corr3D via TensorEngine block-Grams (v9: 4-way col quarters, 144-col).

Per core (20 h-rows of one b): blocks of q voxels (Ah,Aw,At)=(4,8,4) -> M=128.
Each block's matmul splits into 8 concurrent PE tiles tile_position=
(32g, 64*jh) (32x64 mode; 4x col tiling is avoided -- col quadrant 3 is
broken on trn2). Tile (g, jh) computes the jh-th w-half of the block
(voxels with w-in-block jw in [4jh, 4jh+4)) against the w-shifted window
sw in [4jh, 4jh+6), a (6,6,6) = 216-col neighborhood: the minimal
2-way-split window. Output per voxel: 216 cols (v2: 360, v3b: 240); k
duplication drops from 2.25x to 1.875x. Raw tiles stream to DRAM; the host
extracts the 27 banded diagonals with as_strided views.

Block/bookkeeping layout:
  p60 = hb*12 + wb   (hb<5, wb<12);  g = p60 % 4, j = p60 // 4
  qd[j, 32g+c, tb*128 + m],  m = jh*64 + i*16 + jw*4 + l  (jh<2,i<4,jw<4,l<4)
  kd[j, 32g+c, sh*1140 + sw*114 + t]   (sh<6, sw<10, t<114 padded)
  raw[j, tb, m, g*216 + n],  n = (i+dh)*36 + (jw+dw)*6 + (l+dt)
"""

import sys

if "/opt/trn_rl_repo" not in sys.path:
    sys.path.insert(0, "/opt/trn_rl_repo")

import numpy as np
import ml_dtypes

B, C, H, W, T = 2, 32, 80, 96, 112
N_CORES = 8
HR = 20          # h rows per core
HB, WB, TB = 5, 12, 28   # blocks per core along h, w, t
AH, AW, AT = 4, 8, 4     # block shape (M = 128)
SH, SW, ST = 6, 10, 6    # block neighborhood (quarter window is 6x4x6)
NS = 6 * 4 * 6           # 144: per-tile window (6 sh, 4 sw, 6 st)
NJ = (HB * WB) // 4      # 15 block-quads per core
QF = TB * 128            # 3584 q elems per (j, partition)
KF = SH * SW * (T + 2)   # 6840 k elems per (j, partition)
RF = 4 * NS              # 576 raw cols per (j, tb, partition)

bf16 = ml_dtypes.bfloat16

_CACHE = {}


def build_nc(n_j=NJ, repeat=1):
    import contextlib
    import concourse.bass as bass  # noqa: F401
    import concourse.tile as tile
    from concourse import bacc, mybir

    dt = mybir.dt
    nc = bacc.Bacc("TRN2", target_bir_lowering=False, debug=False,
                   num_devices=N_CORES)
    q_ext = nc.dram_tensor("q", [n_j, 128, QF], dt.bfloat16,
                           kind="ExternalInput")
    k_ext = nc.dram_tensor("k", [n_j, 128, KF], dt.bfloat16,
                           kind="ExternalInput")
    o_ext = nc.dram_tensor("o", [n_j, TB, 128, RF], dt.bfloat16,
                           kind="ExternalOutput")

    with tile.TileContext(nc) as tc:
        with (
            tc.For_i(0, repeat, 1) if repeat > 1
            else contextlib.nullcontext(),
            tc.tile_pool(name="kpool", bufs=3) as kpool,
            tc.tile_pool(name="qpool", bufs=2) as qpool,
            tc.tile_pool(name="spool", bufs=4) as spool,
            tc.tile_pool(name="psum", bufs=2, space="PSUM") as pspool,
        ):
            tiles = {}

            def load(j):
                kt = kpool.tile([128, KF], dt.bfloat16)
                nc.sync.dma_start(kt[:], k_ext[j])
                qt = qpool.tile([128, QF], dt.bfloat16)
                nc.sync.dma_start(qt[:], q_ext[j])
                tiles[j] = (kt, qt)

            load(0)
            for j in range(n_j):
                if j + 1 < n_j:
                    load(j + 1)
                kt, qt = tiles.pop(j)
                k4 = kt[:].rearrange("p (sh sw t) -> p sh sw t",
                                     sh=SH, sw=SW, t=T + 2)
                for tb0 in range(0, TB, 4):
                    st = spool.tile([128, 4 * RF], dt.bfloat16)
                    for tb in range(tb0, tb0 + 4):
                        # 4-bank PSUM region; tile (g, jh) -> bank g, half jh
                        ps = pspool.tile([128, 4 * 512], dt.float32)
                        for qd in range(4):
                            for g in range(4):
                                lhsT = qt[32 * g:32 * (g + 1),
                                          tb * 128 + 32 * qd:
                                          tb * 128 + 32 * (qd + 1)]
                                rhs = k4[32 * g:32 * (g + 1), :,
                                         2 * qd:2 * qd + 4,
                                         tb * AT:tb * AT + 6]
                                nc.tensor.matmul(
                                    ps[32 * qd:32 * (qd + 1),
                                       g * 512:g * 512 + NS],
                                    lhsT, rhs,
                                    tile_position=(32 * g, 32 * qd))
                        # single strided eviction of four banks, f32 -> bf16
                        ps4 = ps[:].rearrange("p (g n) -> p g n", g=4, n=512)
                        sl = st[:, (tb - tb0) * RF:(tb - tb0 + 1) * RF]
                        st4 = sl.rearrange("p (g n) -> p g n", g=4, n=NS)
                        nc.vector.tensor_copy(st4[:, 0:2],
                                              ps4[:, 0:2, 0:NS])
                        nc.scalar.copy(st4[:, 2:4], ps4[:, 2:4, 0:NS])
                    nc.sync.dma_start(
                        o_ext[j][tb0:tb0 + 4].rearrange("tb p f -> p tb f"),
                        st[:].rearrange("p (tb f) -> p tb f", tb=4, f=RF))
    nc.compile()
    return nc


def prep_inputs(q, k):
    q = np.asarray(q, dtype=np.float32)
    k = np.asarray(k, dtype=np.float32)
    qs = (q * np.float32(1.0 / C)).astype(bf16)
    kpad = np.zeros((B, C, H + 2, W + 2, T + 2), dtype=bf16)
    kpad[:, :, 1:H + 1, 1:W + 1, 1:T + 1] = k.astype(bf16)
    in_maps = []
    for r in range(N_CORES):
        b = r // (N_CORES // B)
        h0 = (r % (N_CORES // B)) * HR
        # q blocks: [p60, c, tb, jh, i, jw, l] -> [j, (g, c), tb*128 + m]
        qb = qs[b, :, h0:h0 + HR]            # (C, 20, 96, 112)
        s_c, s_h, s_w, s_t = qb.strides
        qv = np.lib.stride_tricks.as_strided(
            qb, shape=(HB, WB, C, TB, AH, 4, 2, AT),
            strides=(AH * s_h, AW * s_w, s_c, AT * s_t,
                     s_h, 2 * s_w, s_w, s_t))
        qv = qv.transpose(0, 1, 2, 3, 5, 4, 6, 7)    # (.., qd, i, jwp, l)
        qv = qv.reshape(HB * WB, C, TB, 128)         # [p60, c, tb, m]
        qv = qv.reshape(NJ, 4, C, TB * 128)          # p60 = j*4+g
        q_core = np.ascontiguousarray(qv).reshape(NJ, 128, QF)
        # k slabs: [p60, c, sh, sw, t] (padded windows, stride (4,8) blocks)
        kb = kpad[b, :, h0:h0 + HR + 2]      # (C, 22, 98, 114)
        s_c, s_h, s_w, s_t = kb.strides
        kv = np.lib.stride_tricks.as_strided(
            kb, shape=(HB, WB, C, SH, SW, T + 2),
            strides=(AH * s_h, AW * s_w, s_c, s_h, s_w, s_t))
        kv = kv.reshape(HB * WB, C, KF).reshape(NJ, 4, C, KF)
        k_core = np.ascontiguousarray(kv).reshape(NJ, 128, KF)
        in_maps.append({"q": q_core, "k": k_core})
    return in_maps


def assemble_output(results):
    out = np.empty((B, 27, H, W, T), dtype=np.float32)
    core_out = np.empty((27, HB * WB, AH, AW, TB, AT), dtype=np.float32)
    for r in range(N_CORES):
        b = r // (N_CORES // B)
        h0 = (r % (N_CORES // B)) * HR
        raw = np.asarray(results[r]["o"])            # [NJ, TB, 128, RF] bf16
        flat = raw.reshape(-1)
        sj = TB * 128 * RF
        stb = 128 * RF
        for g in range(4):
            for dh in range(3):
                for dw in range(3):
                    for dtt in range(3):
                        tap = dh * 9 + dw * 3 + dtt
                        off = g * NS + dh * 24 + dw * 6 + dtt
                        view = np.lib.stride_tricks.as_strided(
                            flat[off:],
                            shape=(NJ, TB, 4, AH, 2, AT),
                            strides=tuple(2 * x for x in (
                                sj, stb, 32 * RF, 8 * RF + 24,
                                4 * RF + 6, RF + 1)))
                        # [j, tb, qd, i, jwp, l] -> [j, i, (qd jwp), tb, l]
                        core_out[tap, g::4] = view.transpose(
                            0, 3, 2, 4, 1, 5).reshape(NJ, AH, AW, TB, AT)
        co = core_out.reshape(27, HB, WB, AH, AW, TB, AT)
        co = co.transpose(0, 1, 3, 2, 4, 5, 6)       # tap, hb, i, wb, jw, tb, l
        out[b, :, h0:h0 + HR] = co.reshape(27, HR, W, T)
    return out


def kernel(q, k):
    from concourse.bass_utils import run_bass_kernel_spmd

    if "nc" not in _CACHE:
        _CACHE["nc"] = build_nc()
    nc = _CACHE["nc"]
    in_maps = prep_inputs(q, k)
    try:
        res = run_bass_kernel_spmd(nc, in_maps,
                                   core_ids=list(range(N_CORES)))
    except Exception:
        # rare transient NRT_EXEC_UNIT_UNRECOVERABLE under axon -- retry
        # once on a fresh backend
        import jax
        try:
            jax.clear_caches()
        except Exception:
            pass
        res = run_bass_kernel_spmd(nc, in_maps,
                                   core_ids=list(range(N_CORES)))
    return assemble_output(res.results)
